# revision 2
# baseline (speedup 1.0000x reference)
"""Trainium2 Bass kernel for nn_ConsumptionPredictor.

Model: 2-layer LSTM (H=5, S=512) over batch 16384, then two linear layers
with no intervening nonlinearity (so W1/W2 collapse into a single 2560-dim
dot product v = W2 @ W1, c0 = W2 @ b1 + b2).

Strategy (per core, batch 2048 = 1/8 of 16384):
  * Sequence chunking: C=4 chunks of L=128 steps, each chunk warmed up with
    W=24 extra leading steps (LSTM forget gate ~0.5/step => warmup error
    ~1e-4 relative).  All 4 chunks of both batch halves run in lockstep as
    8 "groups" of 1024 samples -> virtual batch 8192 per step.
  * Both LSTM layers are fused in one step loop (layer 1 lags layer 0 by
    one step), doubling per-instruction work.
  * Per-gate PSUM tiles (80, 1024): rows 0:40 layer-0 (8 groups x 5 hidden),
    rows 40:80 layer-1.  Sigmoid/Tanh on ScalarE with the gate bias folded
    into the activation's per-partition bias operand.
  * Cell update on VectorE; hidden state written straight into the stacked
    matmul-rhs tiles; h0 copied to layer-1's rhs rows by SBUF-to-SBUF DMA.
  * h1 history stored as fp16 in SBUF; readout out[b] = sum_s v[s,:]h1[b,s,:]
    done as a tail phase of small accumulating matmuls with per-step v
    blocks as lhsT.
"""

import sys

import numpy as np

try:
    import concourse.bass as bass  # noqa: F401
except ImportError:  # pragma: no cover
    sys.path.insert(0, "/opt/trn_rl_repo")

import concourse.bass as bass
import concourse.tile as tile
from concourse import bacc, mybir
from concourse.bass_utils import run_bass_kernel_spmd

# ----- problem/config constants (hardcoded; kernel.py must be self-contained)
NCORES = 8
B, S, H = 16384, 512, 5
BC = B // NCORES          # 2048 samples per core
C = 4                     # sequence chunks
L = S // C                # 128 steps per chunk
W = 24                    # warmup steps per chunk
T = W + L + 1             # virtual steps (layer 1 lags by one)
G = 8                     # groups = C * (BC // BF)
BF = 1024                 # free width per group
J = 2                     # column slices for cross-engine pipelining
JS = BF // J              # 512
L1O = 64                  # layer-1 row base in gate/psum tiles (PE needs 0/32/64)
NP = L1O + 40             # tile partition height (104)
NG = 4                    # gate count (i, f, g, o)
GI, GF, GG, GO = 0, 1, 2, 3
F32 = mybir.dt.float32
F16 = mybir.dt.float16

_CACHE = {}


def _build_program():
    """Build + compile the per-core Bass program once."""
    if "nc" in _CACHE:
        return _CACHE["nc"]

    nc = bacc.Bacc("TRN2", target_bir_lowering=False, debug=False,
                   enable_asserts=False, num_devices=NCORES)

    xT = nc.dram_tensor("xT", [W + S, BC], F32, kind="ExternalInput")
    lhsT0_d = nc.dram_tensor("lhsT0", [48, NG * 40], F32, kind="ExternalInput")
    lhsT1_d = nc.dram_tensor("lhsT1", [NP, NG * 40], F32, kind="ExternalInput")
    biasT_d = nc.dram_tensor("biasT", [NP, NG], F32, kind="ExternalInput")
    vtab_d = nc.dram_tensor("vtab", [40, L * G], F32, kind="ExternalInput")
    sel_d = nc.dram_tensor("sel", [8, 2], F32, kind="ExternalInput")
    out_d = nc.dram_tensor("out", [2, BF], F32, kind="ExternalOutput")

    with tile.TileContext(nc) as tc:
        with (
            tc.tile_pool(name="consts", bufs=1) as consts,
            tc.tile_pool(name="state", bufs=1) as state,
            tc.tile_pool(name="work", bufs=2) as work,
            tc.tile_pool(name="xin", bufs=3) as xin,
        ):
            lhsT0 = consts.tile([48, NG * 40], F32)
            nc.sync.dma_start(out=lhsT0[:], in_=lhsT0_d.ap())
            lhsT1 = consts.tile([NP, NG * 40], F32)
            nc.sync.dma_start(out=lhsT1[:], in_=lhsT1_d.ap())
            biasT = consts.tile([NP, NG], F32)
            nc.sync.dma_start(out=biasT[:], in_=biasT_d.ap())
            vtab = consts.tile([40, L * G], F32)
            nc.sync.dma_start(out=vtab[:], in_=vtab_d.ap())
            sel = consts.tile([8, 2], F32)
            nc.sync.dma_start(out=sel[:], in_=sel_d.ap())

            # persistent state
            hs0 = state.tile([48, BF], F32)   # rows 0:40 h0 state, 40:48 x
            hs1 = state.tile([NP, BF], F32)  # rows 0:40 h1 state, 64:104 h0 in
            cst = state.tile([NP, BF], F32)   # c state, L0 rows 0:40, L1 64:104

            nc.vector.memset(hs0[0:40, :], 0.0)
            nc.vector.memset(hs1[:], 0.0)
            nc.vector.memset(cst[:], 0.0)

            with tc.tile_pool(name="gates", bufs=1, space="PSUM") as gp:
                # one 512-wide PSUM bank per gate; js halves ping-pong through
                # it.  acc/out2 take the remaining 4 banks.
                ps = [gp.tile([NP, JS], F32, tag=f"ps{gi}", name=f"ps{gi}")
                      for gi in range(NG)]
                acc = gp.tile([8, BF], F32, tag="acc", name="acc")
                out2 = gp.tile([2, BF], F32, tag="out2", name="out2")
                for gi in range(NG):
                    # init whole tile: dead rows 40:64 stay zero; L1 rows
                    # 64:104 must be defined before the t=0 activations.
                    nc.vector.memset(ps[gi][:], 0.0)

                for t in range(T):
                    run_l0 = t < W + L
                    run_l1 = t >= 1

                    if run_l0:
                        # x rows for this step: xTpad[c*L + t, half*1024 + j]
                        src = bass.AP(tensor=xT.ap().tensor, offset=t * BC,
                                      ap=[[L * BC, C], [BF, 2], [1, BF]])
                        nc.sync.dma_start(out=hs0[40:48, :], in_=src)

                    if t == W:
                        # chunk 0 (groups 0,1) layer-0 state reset
                        nc.vector.memset(hs0[0:10, :], 0.0)
                        nc.vector.memset(cst[0:10, :], 0.0)
                    if t == W + 1:
                        nc.vector.memset(hs1[0:10, :], 0.0)
                        nc.vector.memset(cst[L1O:L1O + 10, :], 0.0)

                    for js in range(J):
                        sl = slice(js * JS, (js + 1) * JS)
                        for gi in range(NG):
                            if run_l0:
                                nc.tensor.matmul(
                                    ps[gi][0:40, :],
                                    lhsT0[:, gi * 40:(gi + 1) * 40],
                                    hs0[:, sl], start=True, stop=True)
                            if run_l1:
                                nc.tensor.matmul(
                                    ps[gi][L1O:L1O + 40, :],
                                    lhsT1[:, gi * 40:(gi + 1) * 40],
                                    hs1[:, sl], start=True, stop=True)

                        si = work.tile([NP, JS], F32, tag="si")
                        sf = work.tile([NP, JS], F32, tag="sf")
                        so = work.tile([NP, JS], F32, tag="so")
                        tg = work.tile([NP, JS], F32, tag="tg")
                        tc_ = work.tile([NP, JS], F32, tag="tc")
                        ig = work.tile([NP, JS], F32, tag="ig")

                        Sig = mybir.ActivationFunctionType.Sigmoid
                        Tanh = mybir.ActivationFunctionType.Tanh
                        nc.scalar.activation(si[:], ps[GI][:], Sig,
                                             bias=biasT[:, GI:GI + 1])
                        nc.scalar.activation(sf[:], ps[GF][:], Sig,
                                             bias=biasT[:, GF:GF + 1])
                        nc.scalar.activation(so[:], ps[GO][:], Sig,
                                             bias=biasT[:, GO:GO + 1])
                        nc.scalar.activation(tg[:], ps[GG][:], Tanh,
                                             bias=biasT[:, GG:GG + 1])

                        nc.vector.tensor_mul(ig[:], si[:], tg[:])
                        nc.vector.tensor_mul(cst[:, sl], sf[:], cst[:, sl])
                        nc.vector.tensor_add(cst[:, sl], cst[:, sl], ig[:])
                        nc.scalar.activation(tc_[:], cst[:, sl], Tanh)

                        if run_l0:
                            nc.vector.tensor_mul(hs0[0:40, sl], so[0:40, :],
                                                 tc_[0:40, :])
                        if run_l1:
                            nc.vector.tensor_mul(
                                hs1[0:40, sl], so[L1O:L1O + 40, :],
                                tc_[L1O:L1O + 40, :])
                            tp = t - (W + 1)
                            if tp >= 0:
                                nc.tensor.matmul(
                                    acc[:, sl],
                                    vtab[:, tp * G:(tp + 1) * G],
                                    hs1[0:40, sl],
                                    start=(tp == 0), stop=(tp == L - 1))

                    if run_l0:
                        # h0 -> layer-1 rhs rows (next step's input)
                        nc.scalar.copy(hs1[L1O:NP, :], hs0[0:40, :])

                # ---- final reduction: out[h*BF+j] = sum_c acc[(c,h), j] ----
                accs = work.tile([8, BF], F32, tag="accs")
                nc.scalar.copy(accs[:], acc[:])
                for js in range(J):
                    sl = slice(js * JS, (js + 1) * JS)
                    nc.tensor.matmul(out2[:, sl], sel[:], accs[:, sl],
                                     start=True, stop=True)
                outsb = work.tile([2, BF], F32, tag="outsb")
                nc.scalar.copy(outsb[:], out2[:])
                nc.sync.dma_start(out=out_d.ap(), in_=outsb[:])

    nc.compile()
    _CACHE["nc"] = nc
    return nc


def _host_prep(inputs):
    """Build per-core input maps (host-side preprocessing)."""
    x = np.ascontiguousarray(inputs["x"].reshape(B, S).astype(np.float32))
    Wih0, Whh0 = np.asarray(inputs["Wih0"], np.float32), np.asarray(inputs["Whh0"], np.float32)
    Wih1, Whh1 = np.asarray(inputs["Wih1"], np.float32), np.asarray(inputs["Whh1"], np.float32)
    b0 = np.asarray(inputs["bih0"], np.float32) + np.asarray(inputs["bhh0"], np.float32)
    b1l = np.asarray(inputs["bih1"], np.float32) + np.asarray(inputs["bhh1"], np.float32)
    W1, b1 = np.asarray(inputs["W1"], np.float32), np.asarray(inputs["b1"], np.float32)
    W2, b2 = np.asarray(inputs["W2"], np.float32), np.asarray(inputs["b2"], np.float32)

    v2d = (W2 @ W1).reshape(S, H).astype(np.float32)
    c0 = float((W2 @ b1 + b2).reshape(-1)[0])

    # lhsT constants. gate order i,f,g,o matches GI..GO indices 0..3.
    lhsT0 = np.zeros((48, NG * 40), np.float32)
    lhsT1 = np.zeros((NP, NG * 40), np.float32)
    biasT = np.zeros((NP, NG), np.float32)
    for gi in range(NG):
        for g in range(G):
            for k in range(H):
                m = gi * 40 + 5 * g + k
                gr = gi * H + k  # PyTorch gate-row index
                lhsT0[5 * g:5 * g + 5, m] = Whh0[gr, :]
                lhsT0[40 + g, m] = Wih0[gr, 0]
                lhsT1[5 * g:5 * g + 5, m] = Whh1[gr, :]
                lhsT1[L1O + 5 * g:L1O + 5 * g + 5, m] = Wih1[gr, :]
                biasT[5 * g + k, gi] = b0[gr]
                biasT[L1O + 5 * g + k, gi] = b1l[gr]

    vtab = np.zeros((40, L * G), np.float32)
    for tp in range(L):
        for g in range(G):
            s = (g // 2) * L + tp
            vtab[5 * g:5 * g + 5, tp * G + g] = v2d[s, :]

    sel = np.zeros((8, 2), np.float32)
    for g in range(G):
        sel[g, g % 2] = 1.0

    in_maps = []
    for core in range(NCORES):
        xc = x[core * BC:(core + 1) * BC, :]          # (2048, 512)
        xTpad = np.zeros((W + S, BC), np.float32)
        xTpad[W:, :] = xc.T                            # (536, 2048)
        in_maps.append({
            "xT": np.ascontiguousarray(xTpad),
            "lhsT0": lhsT0, "lhsT1": lhsT1, "biasT": biasT,
            "vtab": vtab, "sel": sel,
        })
    return in_maps, c0


def _run(nc, in_maps, **kw):
    return run_bass_kernel_spmd(nc, in_maps, core_ids=list(range(NCORES)), **kw)


def kernel(**inputs):
    nc = _build_program()
    in_maps, c0 = _host_prep(inputs)
    res = _run(nc, in_maps)
    out = np.empty((B, 1), np.float32)
    for core in range(NCORES):
        out[core * BC:(core + 1) * BC, 0] = np.asarray(res.results[core]["out"]).reshape(BC) + c0
    return out

